# revision 18
# baseline (speedup 1.0000x reference)
"""PeakDetectionLoss on 8 Trainium2 cores.

Sharding: time axis split into 8 segments (one per core), all 10 signal rows
(5 ppg + 5 rppg) on every core. Host pre-pads 5 samples of -inf at the global
edges and hands each core overlapping [128, 2058] windows per row, so the
width-11 sliding max needs no device halo exchange. Per-row stats (peak count,
signal sum, peak-value sum) are reduced across partitions with a ones-matmul
and across cores with one tiny [1,30] AllReduce; the amplitude threshold is
applied at quarter resolution (aligned 4-blocks hold at most one peak), and
inter-peak gaps are reduced with a pairwise (first,last,sum-recip) tree.
Host stitches the 1024 strip summaries per row.

Work is spread across the vector (DVE), gpsimd (Pool) and scalar (Act)
engines so no single engine is the bottleneck. `_build_program(krepeat=K)`
unrolls the body K times — used by test.py to measure the on-device
execution time differentially (network RTT cancels in the K-vs-1 delta).
"""
import os
import sys

for _p in ("/opt/trn_rl_repo", "/root/.axon_site/_ro/trn_rl_repo"):
    if _p not in sys.path:
        sys.path.append(_p)

import numpy as np

N = 5
L = 2097152
C = 8
SEG = L // C            # 262144
P = 128
PW = SEG // P           # 2048
NB = PW // 4            # 512
TILE_W = PW + 10        # 2058
R = 2 * N               # 10 rows per core
BIG = np.float32(1.0e30)

_STATE = {}


def _build_program(krepeat=1):
    from concourse import bacc, tile, mybir
    from concourse.alu_op_type import AluOpType as op

    f32 = mybir.dt.float32
    nc = bacc.Bacc("TRN2", target_bir_lowering=False, debug=False, num_devices=C)

    xin = nc.dram_tensor("xin", [R, P, TILE_W], f32, kind="ExternalInput")
    bidx = nc.dram_tensor("bidx", [P, NB], f32, kind="ExternalInput")
    summ = nc.dram_tensor("summ", [P, 40], f32, kind="ExternalOutput")

    with tile.TileContext(nc) as tc:
        with (
            tc.tile_pool(name="sb", bufs=1) as sb,
            tc.tile_pool(name="dram", bufs=1, space="DRAM") as dram,
            tc.tile_pool(name="ps", bufs=1, space="PSUM") as ps,
        ):
            bidx_sb = sb.tile([P, NB], f32, tag="bidx")
            ones = sb.tile([P, 1], f32, tag="ones")
            stats = sb.tile([P, 3 * R], f32, tag="stats")
            B4m = sb.tile([P, R * NB], f32, tag="B4m")
            h = sb.tile([P, 2 * R * NB], f32, tag="h")
            treeB = sb.tile([P, 2 * R * 256], f32, tag="treeB")
            gapb = sb.tile([P, R * 256], f32, tag="gapb")
            tbc = sb.tile([P, R], f32, tag="tbc")
            arsb = sb.tile([1, 3 * R], f32, tag="arsb")
            arst = sb.tile([1, 3 * R], f32, tag="arst")
            trec = sb.tile([1, R], f32, tag="trec")
            tmean = sb.tile([1, R], f32, tag="tmean")
            tthr = sb.tile([1, R], f32, tag="tthr")
            summ_sb = sb.tile([P, 40], f32, tag="summ_sb")

            ar_in = dram.tile([1, 3 * R], f32)
            ar_out = dram.tile([1, 3 * R], f32)
            psum_t = ps.tile([1, 3 * R], f32)

            xin_ap = xin.ap()
            nc.sync.dma_start(bidx_sb, bidx.ap())
            nc.vector.memset(ones, 1.0)

            for rep in range(krepeat):
                for r in range(R):
                    # per-row working tiles, double-buffered to decouple
                    # consecutive rows (cross-engine WAR would serialize)
                    xt = sb.tile([P, TILE_W], f32, tag="xt", bufs=2,
                                 name=f"xt{rep}_{r}")
                    M2 = sb.tile([P, 2057], f32, tag="M2", bufs=2,
                                 name=f"M2_{rep}_{r}")
                    M4 = sb.tile([P, 2055], f32, tag="M4", bufs=2,
                                 name=f"M4_{rep}_{r}")
                    Wt = sb.tile([P, PW], f32, tag="Wt", bufs=2,
                                 name=f"Wt{rep}_{r}")
                    m1 = sb.tile([P, PW], f32, tag="m1", bufs=2,
                                 name=f"m1_{rep}_{r}")
                    jk = sb.tile([P, PW], f32, tag="jk", bufs=1,
                                 name=f"jk{rep}_{r}")
                    n1 = sb.tile([P, NB], f32, tag="n1", bufs=2,
                                 name=f"n1_{rep}_{r}")
                    aB = sb.tile([P, NB], f32, tag="aB", bufs=1,
                                 name=f"aB{rep}_{r}")
                    bn = sb.tile([P, NB], f32, tag="bn", bufs=1,
                                 name=f"bn{rep}_{r}")
                    p1 = sb.tile([P, NB], f32, tag="p1", bufs=1,
                                 name=f"p1_{rep}_{r}")
                    p2 = sb.tile([P, NB], f32, tag="p2", bufs=1,
                                 name=f"p2_{rep}_{r}")

                    nc.sync.dma_start(xt, xin_ap[r])

                    # sliding max chain (window 11, centered at xt[:, j+5]);
                    # fp32 tensor_tensor only runs on vector (Pool ISA
                    # rejects max), at 1x mode (no 2x uop for fp32 TT)
                    nc.vector.tensor_tensor(
                        out=M2, in0=xt[:, 0:2057], in1=xt[:, 1:2058], op=op.max)
                    nc.vector.tensor_tensor(
                        out=M4, in0=M2[:, 0:2055], in1=M2[:, 2:2057], op=op.max)
                    M8 = M2[:, 0:2051]  # M2 storage reused for M8
                    nc.vector.tensor_tensor(
                        out=M8, in0=M4[:, 0:2051], in1=M4[:, 4:2055], op=op.max)
                    nc.vector.tensor_tensor(
                        out=Wt, in0=M8[:, 0:PW], in1=M2[:, 3:3 + PW], op=op.max)

                    xc = xt[:, 5:5 + PW]
                    # m1 = (x == window max); accum -> per-partition peak count
                    # (ttr crashes the device at runtime; stt+accum works)
                    nc.vector.scalar_tensor_tensor(
                        out=m1, in0=xc, scalar=0.0, op0=op.bypass,
                        in1=Wt, op1=op.is_ge,
                        accum_out=stats[:, 3 * r:3 * r + 1])
                    # Sx on ScalarE (free accumulate)
                    nc.scalar.activation(
                        out=jk, in_=xc, func=mybir.ActivationFunctionType.Copy,
                        accum_out=stats[:, 3 * r + 1:3 * r + 2])
                    # peaks per aligned 4-block (0 or 1 each)
                    nc.vector.tensor_reduce(
                        out=n1, in_=m1.rearrange("p (b k) -> p b k", k=4),
                        axis=mybir.AxisListType.X, op=op.add)

                    B4 = M4[:, 5:2052:4]  # aligned block-4 max, free view of M4
                    # bn = B4*n1 (fused accum -> sum of peak values)
                    nc.vector.scalar_tensor_tensor(
                        out=bn, in0=B4, scalar=0.0, op0=op.bypass, in1=n1,
                        op1=op.mult, accum_out=stats[:, 3 * r + 2:3 * r + 3])
                    # aB = 0 if peak else -BIG (Act: copy with scale/bias)
                    nc.scalar.activation(
                        out=aB, in_=n1, func=mybir.ActivationFunctionType.Copy,
                        scale=float(BIG), bias=float(-BIG))
                    nc.gpsimd.tensor_tensor(
                        out=B4m[:, r * NB:(r + 1) * NB], in0=bn, in1=aB,
                        op=op.add)

                    # peak slot inside block from m1 (<=1 peak per 4-block)
                    nc.vector.scalar_tensor_tensor(
                        out=p1, in0=m1[:, 2:2048:4], scalar=2.0, op0=op.mult,
                        in1=m1[:, 1:2048:4], op1=op.add)
                    nc.vector.scalar_tensor_tensor(
                        out=p2, in0=m1[:, 3:2048:4], scalar=3.0, op0=op.mult,
                        in1=p1, op1=op.add)
                    pos_row = h[:, r * NB:(r + 1) * NB]
                    neg_row = h[:, (R + r) * NB:(R + r + 1) * NB]
                    nc.gpsimd.tensor_tensor(
                        out=pos_row, in0=p2, in1=bidx_sb, op=op.add)
                    nc.scalar.mul(neg_row, pos_row, -1.0)

                # cross-partition stats reduce, then cross-core AllReduce
                nc.tensor.matmul(
                    out=psum_t[0:1, :], lhsT=ones, rhs=stats,
                    start=True, stop=True)
                nc.scalar.copy(arst, psum_t[0:1, :])
                nc.sync.dma_start(ar_in, arst)
                nc.gpsimd.collective_compute(
                    "AllReduce", op.add, replica_groups=[list(range(C))],
                    ins=[ar_in.opt()], outs=[ar_out.opt()])
                nc.sync.dma_start(arsb, ar_out)

                # threshold t_r = Sx_r/(2L) + 0.5*sv_r/npk_r (raw-space, exact)
                a_npk = arsb[0:1, 0:3 * R:3]
                a_sx = arsb[0:1, 1:3 * R:3]
                a_sv = arsb[0:1, 2:3 * R:3]
                nc.vector.reciprocal(out=trec[0:1, :], in_=a_npk)
                nc.vector.scalar_tensor_tensor(
                    out=tmean[0:1, :], in0=trec[0:1, :], scalar=0.5,
                    op0=op.mult, in1=a_sv, op1=op.mult)
                nc.vector.scalar_tensor_tensor(
                    out=tthr[0:1, :], in0=a_sx, scalar=0.5 / L, op0=op.mult,
                    in1=tmean[0:1, :], op1=op.add)
                nc.gpsimd.partition_broadcast(tbc, tthr[0:1, :])

                # phase 2: amplitude filter at quarter res + pairwise gap tree
                # (notv lives in treeB's storage until level 1 overwrites it;
                # the reciprocal scratch lives in B4m's storage, dead after
                # the notv compare; h doubles as the second tree buffer)
                # Per-row tensor_scalar (single-source, 2x-mode eligible)
                # fuses the threshold compare with the block count via
                # accum_out; the 0/1 mask is then scaled to 0/BIG in place
                # and added to both tree inputs (Pool takes one half).
                B4m_v = B4m.rearrange("p (r b) -> p r b", r=R)
                notv = treeB.rearrange("p (r b) -> p r b", r=R)
                notv_f = treeB[:, 0:R * NB]
                h4 = h.rearrange("p (a r b) -> p a r b", a=2, r=R)
                for r in range(R):
                    nc.vector.tensor_scalar(
                        out=notv[:, r], in0=B4m_v[:, r],
                        scalar1=tbc[:, r:r + 1], scalar2=0.0, op0=op.is_le,
                        op1=op.add, accum_out=summ_sb[:, 30 + r:31 + r])
                nc.vector.tensor_scalar(
                    out=notv_f, in0=notv_f, scalar1=float(BIG), scalar2=None,
                    op0=op.mult)
                # mask sub-threshold blocks to +BIG in both tree inputs
                # (pos <= L << eps(BIG), so pos + BIG == BIG exactly)
                nc.vector.tensor_tensor(
                    out=h4[:, 0], in0=h4[:, 0], in1=notv, op=op.add)
                nc.gpsimd.tensor_tensor(
                    out=h4[:, 1], in0=h4[:, 1], in1=notv, op=op.add)

                treeB4 = treeB.rearrange("p (a r c) -> p a r c", a=2, r=R)
                gap4 = gapb.rearrange("p (x r c) -> p x r c", x=1, r=R)
                scr4 = B4m.rearrange("p (x r c) -> p x r c", x=1, r=R)
                summ_h = (summ_sb[:, 0:2 * R]
                          .rearrange("p (a r) -> p a r", a=2).unsqueeze(-1))

                cur = h4
                w = NB
                off = 0
                bufs_cycle = [treeB4, h4]
                lvl = 0
                while w > 1:
                    w2 = w // 2
                    out_h = (summ_h if w2 == 1
                             else bufs_cycle[lvl % 2][:, :, :, 0:w2])
                    nc.vector.tensor_tensor(
                        out=out_h, in0=cur[:, :, :, 0:w:2],
                        in1=cur[:, :, :, 1:w:2], op=op.min)
                    g = gap4[:, :, :, 0:w2]
                    nc.gpsimd.tensor_tensor(
                        out=g, in0=cur[:, 0:1, :, 1:w:2],
                        in1=cur[:, 1:2, :, 0:w:2], op=op.add)
                    nc.vector.reciprocal(out=scr4[:, :, :, off:off + w2], in_=g)
                    off += w2
                    cur = out_h
                    w = w2
                    lvl += 1

                nc.vector.tensor_reduce(
                    out=summ_sb[:, 20:30], in_=scr4[:, 0, :, 0:511],
                    axis=mybir.AxisListType.X, op=op.add)
                nc.sync.dma_start(summ.ap(), summ_sb)

    nc.compile()
    return nc


def _get_runner(krepeat=1):
    """Build once per krepeat; return fn(in_maps) -> per-core {name: arr}."""
    key = ("runner", krepeat)
    if key in _STATE:
        return _STATE[key]

    import jax
    from jax.sharding import Mesh, PartitionSpec
    from jax.experimental.shard_map import shard_map
    from concourse import bass2jax, mybir

    nc = _build_program(krepeat=krepeat)
    bass2jax.install_neuronx_cc_hook()

    partition_name = (
        nc.partition_id_tensor.name if nc.partition_id_tensor else None
    )
    in_names, out_names, out_avals, zero_outs = [], [], [], []
    for alloc in nc.m.functions[0].allocations:
        if not isinstance(alloc, mybir.MemoryLocationSet):
            continue
        name = alloc.memorylocations[0].name
        if alloc.kind == "ExternalInput":
            if name != partition_name:
                in_names.append(name)
        elif alloc.kind == "ExternalOutput":
            out_names.append(name)
            shape = tuple(alloc.tensor_shape)
            dtype = mybir.dt.np(alloc.dtype)
            out_avals.append(jax.core.ShapedArray(shape, dtype))
            zero_outs.append(np.zeros(shape, dtype))
    n_params = len(in_names)
    n_outs = len(out_avals)
    all_names = in_names + out_names
    if partition_name is not None:
        all_names = all_names + [partition_name]

    def _body(*args):
        operands = list(args)
        if partition_name is not None:
            operands.append(bass2jax.partition_id_tensor())
        outs = bass2jax._bass_exec_p.bind(
            *operands,
            out_avals=tuple(out_avals),
            in_names=tuple(all_names),
            out_names=tuple(out_names),
            lowering_input_output_aliases=(),
            sim_require_finite=False,
            sim_require_nnan=False,
            nc=nc,
        )
        return tuple(outs)

    devices = jax.devices()[:C]
    assert len(devices) == C, f"need {C} devices, have {len(jax.devices())}"
    mesh = Mesh(np.asarray(devices), ("core",))
    donate = tuple(range(n_params, n_params + n_outs))
    sharded = jax.jit(
        shard_map(
            _body, mesh=mesh,
            in_specs=(PartitionSpec("core"),) * (n_params + n_outs),
            out_specs=(PartitionSpec("core"),) * n_outs,
            check_rep=False,
        ),
        donate_argnums=donate,
        keep_unused=True,
    )

    def run(in_maps):
        concat_in = [
            np.concatenate([np.asarray(m[nm]) for m in in_maps], axis=0)
            for nm in in_names
        ]
        concat_zeros = [
            np.zeros((C * z.shape[0], *z.shape[1:]), z.dtype) for z in zero_outs
        ]
        out_arrs = sharded(*concat_in, *concat_zeros)
        return [
            {nm: np.asarray(out_arrs[i]).reshape(C, *out_avals[i].shape)[c]
             for i, nm in enumerate(out_names)}
            for c in range(C)
        ]

    run.in_names = in_names
    run.out_names = out_names
    run.sharded = sharded
    run.zero_outs = zero_outs
    _STATE[key] = run
    return run


def make_in_maps(rppg, ppg):
    sigs = np.concatenate(
        [np.asarray(ppg, np.float32).reshape(N, L),
         np.asarray(rppg, np.float32).reshape(N, L)], axis=0)
    padded = np.full((R, L + 10), -np.inf, np.float32)
    padded[:, 5:5 + L] = sigs
    win = np.lib.stride_tricks.sliding_window_view(padded, TILE_W, axis=1)
    in_maps = []
    prow = np.arange(P, dtype=np.float32)[:, None] * PW
    brow = np.arange(NB, dtype=np.float32)[None, :] * 4.0
    for c in range(C):
        xin_c = np.ascontiguousarray(win[:, c * SEG:c * SEG + SEG:PW, :])
        bidx_c = (np.float32(c * SEG) + prow + brow).astype(np.float32)
        in_maps.append({"xin": xin_c, "bidx": bidx_c})
    return in_maps


def stitch(results, fs):
    summ = np.stack([results[c]["summ"] for c in range(C)])  # [C, 128, 40]
    hr = np.zeros(R)
    for r in range(R):
        f = summ[:, :, r].reshape(-1).astype(np.float64)
        g = -summ[:, :, R + r].reshape(-1).astype(np.float64)
        s = summ[:, :, 2 * R + r].astype(np.float64).sum()
        n = (512.0 - summ[:, :, 3 * R + r].astype(np.float64)).sum()
        ne = f < float(BIG) / 2
        fs_, gs_ = f[ne], g[ne]
        s += (1.0 / (fs_[1:] - gs_[:-1])).sum()
        hr[r] = 60.0 * float(fs) * s / (n - 1.0)
    return np.float32(np.mean(np.abs(hr[0:N] - hr[N:R]) / hr[0:N]))


def kernel(rppg, ppg, fs, epoch):
    run = _get_runner()
    results = run(make_in_maps(rppg, ppg))
    return stitch(results, fs)
